# revision 20
# baseline (speedup 1.0000x reference)
"""Chamfer loss kernel for TRN2 (8 NeuronCores, data-parallel over batch).

Reference computation (per batch b):
  t = l2_normalize(tokens[b])      # (K=1024, D=128)
  i = l2_normalize(interests[b])   # (M=64,  D=128)
  dist[k,m] = sqrt(2 - 2*dot(t_k, i_m))   (since ||t||=||i||=1)
  loss = mean_bm(min_k dist) + 0.3 * mean_bk(min_m dist)

min dist  <=>  max dot, so we reduce max over dots and apply
sqrt(2-2x) only to the tiny reduced tensors.

Each core processes 64 batches and emits two partial sums
(sum of per-token min dists, sum of per-interest min dists);
the host combines them into the final scalar.
"""

import os
import numpy as np
from contextlib import ExitStack

import concourse.bass as bass
import concourse.mybir as mybir
import concourse.tile as tile
from concourse import bacc
from concourse.bass_utils import run_bass_kernel_spmd
from concourse.masks import make_identity

N_CORES = 8
B, K, M, D = 512, 1024, 64, 128
B_LOC = B // N_CORES          # 64 batches per core
KT = K // 128                 # 8 token tiles of [128, D] per batch
ALPHA_T_TO_I = 0.3

F32 = mybir.dt.float32
F32R = mybir.dt.float32r
AX = mybir.AxisListType
OP = mybir.AluOpType
ACT = mybir.ActivationFunctionType

# float32r: the PE's fast fp32 mode (1 cycle/row for wide matmuls vs 4 for
# plain fp32). Producers of matmul operands must emit float32r-rounded
# values, so the whole matmul-operand path uses MMDT as its dtype.
USE_F32R = True
MMDT = F32R if USE_F32R else F32


def build(b_loc=B_LOC, stages=3):
    nc = bacc.Bacc(
        "TRN2",
        target_bir_lowering=False,
        debug=False,
        num_devices=N_CORES,
    )
    tokens = nc.dram_tensor("tokens", [b_loc, K, D], F32, kind="ExternalInput").ap()
    interests = nc.dram_tensor(
        "interests", [b_loc, M, D], F32, kind="ExternalInput"
    ).ap()
    out = nc.dram_tensor("out", [1, 2], F32, kind="ExternalOutput").ap()

    with ExitStack() as ctx:
        tc = ctx.enter_context(tile.TileContext(nc))
        singles = ctx.enter_context(tc.tile_pool(name="singles", bufs=1))
        tok_pool = ctx.enter_context(tc.tile_pool(name="tok", bufs=2))
        work = ctx.enter_context(tc.tile_pool(name="work", bufs=2))
        small = ctx.enter_context(tc.tile_pool(name="small", bufs=3))
        p_tT = ctx.enter_context(tc.tile_pool(name="p_tT", bufs=2, space="PSUM"))
        p_dots = ctx.enter_context(tc.tile_pool(name="p_dots", bufs=2, space="PSUM"))
        p_dT = ctx.enter_context(tc.tile_pool(name="p_dT", bufs=1, space="PSUM"))
        p_iT = ctx.enter_context(tc.tile_pool(name="p_iT", bufs=2, space="PSUM"))

        identity = singles.tile([128, 128], MMDT)
        nc.gpsimd.memset(identity.bitcast(mybir.dt.uint32), 0)
        nc.gpsimd.affine_select(
            out=identity, in_=identity, compare_op=OP.not_equal, fill=1.0,
            base=0, pattern=[[-1, 128]], channel_multiplier=1,
        )
        ones = singles.tile([128, 1], F32)
        nc.vector.memset(ones, 1.0)
        two = singles.tile([128, 1], F32)
        nc.vector.memset(two, 2.0)
        acc_t = singles.tile([128, KT], F32)   # per-(p,n)=per-token sums
        acc_i = singles.tile([128, 1], F32)    # per-interest sums (pair-stacked)
        accs = singles.tile([128, 2], F32)
        nc.vector.memset(acc_t, 0.0)
        nc.vector.memset(acc_i, 0.0)

        for b in range(b_loc):
            if True:
                t_all = tok_pool.tile([128, KT, D], F32)
                nc.sync.dma_start(
                    out=t_all, in_=tokens[b].rearrange("(n p) d -> p n d", p=128)
                )
                i_nat = small.tile([M, D], F32, tag="i_nat")
                nc.sync.dma_start(out=i_nat, in_=interests[b])

                # ---- token norms ----
                t2 = work.tile([128, KT, D], F32, tag="t2")
                nc.vector.tensor_mul(t2, t_all, t_all)
                tsum = small.tile([128, KT], F32, tag="tsum")
                nc.vector.tensor_reduce(tsum, t2, axis=AX.X, op=OP.add)
                tnrm = small.tile([128, KT], F32, tag="tnrm")
                nc.scalar.sqrt(tnrm, tsum)
                invt = small.tile([128, KT], F32, tag="invt")
                nc.vector.reciprocal(invt, tnrm)
                tn = work.tile([128, KT, D], MMDT, tag="tn")
                nc.vector.tensor_mul(tn, t_all, invt.broadcast_to([128, KT, D]))

                # ---- interest norms ----
                isq = small.tile([M, D], F32, tag="isq")
                nc.vector.tensor_mul(isq, i_nat, i_nat)
                isum = small.tile([M, 1], F32, tag="isum")
                nc.vector.tensor_reduce(isum, isq, axis=AX.X, op=OP.add)
                inrm = small.tile([M, 1], F32, tag="inrm")
                nc.scalar.sqrt(inrm, isum)
                invi = small.tile([M, 1], F32, tag="invi")
                nc.vector.reciprocal(invi, inrm)
                i_n = small.tile([M, D], MMDT, tag="i_n")
                nc.vector.tensor_scalar_mul(i_n, i_nat, invi)

                if stages < 2:
                    continue
                # ---- transpose normalized interests: iT[d, m] ----
                piT = p_iT.tile([128, M], MMDT, tag="piT")
                nc.tensor.transpose(piT, i_n, identity[:M, :M])
                iT = small.tile([128, M], MMDT, tag="iT")
                nc.scalar.copy(iT, piT)

                # ---- transpose normalized tokens: tT[d, (n k)] ----
                tT = work.tile([128, KT, 128], MMDT, tag="tT")
                for h in range(2):
                    ptT = p_tT.tile([128, 512], MMDT)
                    for j in range(4):
                        n = 4 * h + j
                        nc.tensor.transpose(
                            ptT[:, 128 * j:128 * (j + 1)],
                            tn[:, n, :],
                            identity,
                        )
                    dst = tT[:, 4 * h:4 * (h + 1), :].rearrange("p a b -> p (a b)")
                    if h == 0:
                        nc.vector.tensor_copy(dst, ptT)
                    else:
                        nc.scalar.copy(dst, ptT)

                if stages < 3:
                    continue
                # ---- dots [k, m] orientation: 8 narrow matmuls ----
                pdots = p_dots.tile([128, KT, M], F32)
                for n in range(KT):
                    nc.tensor.matmul(
                        pdots[:, n, :], lhsT=tT[:, n, :], rhs=iT,
                        start=True, stop=True,
                    )

                # ---- dots [m, k] orientation: 2 wide matmuls ----
                img = small.tile([M, 2], F32, tag="img")
                for g in range(2):
                    pdTg = p_dT.tile([M, 512], F32, tag="pdT")
                    nc.tensor.matmul(
                        pdTg,
                        lhsT=iT,
                        rhs=tT[:, 4 * g:4 * (g + 1), :].rearrange("p a b -> p (a b)"),
                        start=True, stop=True,
                    )
                    nc.vector.tensor_reduce(
                        img[:, g:g + 1], pdTg, axis=AX.X, op=OP.max
                    )

                # ---- per-token min dist: max over m, then sqrt(2-2x) ----
                dmax = small.tile([128, KT], F32, tag="dmax")
                nc.vector.tensor_reduce(dmax, pdots, axis=AX.X, op=OP.max)
                dtmin = small.tile([128, KT], F32, tag="dtmin")
                nc.scalar.activation(dtmin, dmax, ACT.Sqrt, bias=two[:], scale=-2.0)
                nc.vector.tensor_add(acc_t, acc_t, dtmin)

                # ---- per-interest min dist ----
                imax = small.tile([M, 1], F32, tag="imax")
                nc.vector.tensor_reduce(imax, img, axis=AX.X, op=OP.max)
                dimin = small.tile([M, 1], F32, tag="dimin")
                nc.scalar.activation(dimin, imax, ACT.Sqrt, bias=two[:M], scale=-2.0)
                nc.vector.tensor_add(acc_i[:M], acc_i[:M], dimin)

        # ---- final partition reduction via ones-matmul ----
        nc.vector.tensor_reduce(accs[:, 0:1], acc_t, axis=AX.X, op=OP.add)
        nc.vector.tensor_copy(accs[:, 1:2], acc_i)
        pfin = p_iT.tile([128, M], F32, tag="piT")
        nc.tensor.matmul(pfin[:1, :2], lhsT=ones, rhs=accs, start=True, stop=True)
        out_sb = small.tile([1, 2], F32, tag="out_sb")
        nc.scalar.copy(out_sb, pfin[:1, :2])
        nc.sync.dma_start(out=out, in_=out_sb)

    nc.compile()
    return nc


_NC_CACHE = None


def _get_nc():
    global _NC_CACHE
    if _NC_CACHE is None:
        _NC_CACHE = build()
    return _NC_CACHE


def kernel(tokens: np.ndarray, interests: np.ndarray, _trace=False) -> np.ndarray:
    tokens = np.ascontiguousarray(tokens, dtype=np.float32)
    interests = np.ascontiguousarray(interests, dtype=np.float32)
    assert tokens.shape == (B, K, D) and interests.shape == (B, M, D)

    nc = _get_nc()
    in_maps = [
        {
            "tokens": tokens[c * B_LOC:(c + 1) * B_LOC],
            "interests": interests[c * B_LOC:(c + 1) * B_LOC],
        }
        for c in range(N_CORES)
    ]
    res = run_bass_kernel_spmd(
        nc, in_maps, core_ids=list(range(N_CORES)), trace=_trace
    )
    sum_t = 0.0  # sum over all (b, k) of min_m dist
    sum_i = 0.0  # sum over all (b, m) of min_k dist
    for r in res.results:
        sum_t += float(r["out"][0, 0])
        sum_i += float(r["out"][0, 1])
    loss = sum_i / (B * M) + ALPHA_T_TO_I * sum_t / (B * K)
    kernel.last_results = res
    return np.array(loss, dtype=np.float32)


# revision 41
# speedup vs baseline: 1.0007x; 1.0007x over previous
"""Chamfer loss kernel for TRN2 (8 NeuronCores, data-parallel over batch).

Reference computation (per batch b):
  t = l2_normalize(tokens[b])      # (K=1024, D=128)
  i = l2_normalize(interests[b])   # (M=64,  D=128)
  dist[k,m] = sqrt(2 - 2*dot(t_k, i_m))   (since ||t||=||i||=1)
  loss = mean_bm(min_k dist) + 0.3 * mean_bk(min_m dist)

min dist <=> max dot: reduce max over normalized dots, apply sqrt(2-2x)
only to tiny reduced tensors.

Structure (per core, 64 batches):
  phase 0: bulk-normalize+transpose ALL interests -> persistent iT_all
  per batch:
    DMA  tokens[b] -> t_all [128,(8,128)]
    PE   8 transposes (raw) -> psum -> copies -> tT [128d,(8n,128k)]
    PE   8 dots matmuls: pdots[k, (n m)] = tT.T @ iT_b
    ACT  tT2 = tT^2 (one big op)
    POOL partition-add over d -> sumsq per token (free-indexed row)
    DMA  4KB layout-gather: row -> tsum [128,8] (partition-indexed)
    ACT  sqrt, DVE reciprocal -> invt [128,8]
    DVE  dn = pdots * invt (fused normalize + psum evacuation)
    DVE  max over m -> staged [128,8]; max over n -> nmax [128,64]
    POOL partition-max over token partitions -> staged per-interest max
    every 4 batches: ACT sqrt(2-2x) on staged maxes, POOL accumulate
Host combines the 8 per-core partial sums.
"""

import os
import numpy as np
from contextlib import ExitStack

import concourse.bass as bass
import concourse.bass_isa as bass_isa
import concourse.mybir as mybir
import concourse.tile as tile
from concourse import bacc
from concourse.bass_utils import run_bass_kernel_spmd

N_CORES = 8
B, K, M, D = 512, 1024, 64, 128
B_LOC = B // N_CORES          # 64 batches per core
KT = K // 128                 # 8 token tiles of [128, D] per batch
ALPHA_T_TO_I = 0.3
STG = 4                       # sqrt-staging factor (batches per sqrt op)

F32 = mybir.dt.float32
AX = mybir.AxisListType
OP = mybir.AluOpType
ACT = mybir.ActivationFunctionType
RED = bass_isa.ReduceOp


def build(b_loc=B_LOC, reps=1):
    assert b_loc % 2 == 0 and b_loc % STG == 0
    nc = bacc.Bacc(
        "TRN2",
        target_bir_lowering=False,
        debug=False,
        num_devices=N_CORES,
    )
    tokens = nc.dram_tensor("tokens", [b_loc, K, D], F32, kind="ExternalInput").ap()
    interests = nc.dram_tensor(
        "interests", [b_loc, M, D], F32, kind="ExternalInput"
    ).ap()
    out = nc.dram_tensor("out", [1, 2], F32, kind="ExternalOutput").ap()

    NG = b_loc * M // 128     # interest row-groups of 128 (b_loc/2)

    with ExitStack() as ctx:
        tc = ctx.enter_context(tile.TileContext(nc))
        singles = ctx.enter_context(tc.tile_pool(name="singles", bufs=1))
        tok_pool = ctx.enter_context(tc.tile_pool(name="tok", bufs=4))
        work = ctx.enter_context(tc.tile_pool(name="work", bufs=4))
        small = ctx.enter_context(tc.tile_pool(name="small", bufs=8))
        stage = ctx.enter_context(tc.tile_pool(name="stage", bufs=3))
        srp = ctx.enter_context(tc.tile_pool(name="srp", bufs=3))
        p_tT = ctx.enter_context(tc.tile_pool(name="p_tT", bufs=2, space="PSUM"))
        p_dots = ctx.enter_context(tc.tile_pool(name="p_dots", bufs=6, space="PSUM"))
        dram = ctx.enter_context(tc.tile_pool(name="dram", bufs=8, space="DRAM"))

        identity = singles.tile([128, 128], F32)
        nc.gpsimd.memset(identity, 0.0)
        nc.gpsimd.affine_select(
            out=identity, in_=identity, compare_op=OP.not_equal, fill=1.0,
            base=0, pattern=[[-1, 128]], channel_multiplier=1,
        )
        ones = singles.tile([128, 1], F32)
        nc.vector.memset(ones, 1.0)
        two = singles.tile([128, 1], F32)
        nc.vector.memset(two, 2.0)
        acc_t = singles.tile([128, STG * KT], F32)
        acc_i = singles.tile([1, STG * M], F32)
        nc.vector.memset(acc_t, 0.0)
        nc.vector.memset(acc_i, 0.0)

        # ---------- phase 0: all interests -> normalized iT_all ----------
        # interests flat (b*M, D) -> groups of 128 rows
        i_flat = interests.rearrange("b m d -> (b m) d").rearrange(
            "(g p) d -> p g d", p=128
        )  # [128, NG, 128]
        i_all = singles.tile([128, NG, D], F32)
        nc.sync.dma_start(out=i_all, in_=i_flat)
        isum = singles.tile([128, NG], F32)
        itrash = work.tile([128, D], F32, tag="trash")
        for g in range(NG):
            nc.scalar.activation(
                itrash, i_all[:, g, :], ACT.Square, accum_out=isum[:, g:g + 1]
            )
        inrm = singles.tile([128, NG], F32)
        nc.scalar.sqrt(inrm, isum)
        invi = singles.tile([128, NG], F32)
        nc.vector.reciprocal(invi, inrm)
        nc.vector.tensor_mul(i_all, i_all, invi.broadcast_to([128, NG, D]))
        iT_all = singles.tile([128, NG, 128], F32)   # [d, (g, bm)]
        for h0 in range(0, NG, 4):
            cn = min(4, NG - h0)
            piT = p_tT.tile([128, 512], F32, tag="ptT")
            for j in range(cn):
                g = h0 + j
                nc.tensor.transpose(
                    piT[:, 128 * j:128 * (j + 1)], i_all[:, g, :], identity
                )
            dst = iT_all[:, h0:h0 + cn, :].rearrange("p a b -> p (a b)")
            if (h0 // 4) % 2 == 0:
                nc.vector.tensor_copy(dst, piT[:, :128 * cn])
            else:
                nc.scalar.copy(dst, piT[:, :128 * cn])

        def iT_of(b):
            # batch b's interests: rows (b*M..b*M+M) = group b//2, half b%2
            return iT_all[:, b // 2, (b % 2) * M:(b % 2) * M + M]

        # ---------- main loop (software-pipelined: tail lags front by STG) ----------
        pdots_of = {}
        invt_of = {}
        st_of = {}
        LAG = 5

        def front(vb):
            b = vb % b_loc
            t_all = tok_pool.tile([128, KT, D], F32)
            nc.sync.dma_start(
                out=t_all, in_=tokens[b].rearrange("(n p) d -> p n d", p=128)
            )

            # transposes of raw token tiles
            tT = work.tile([128, KT, 128], F32, tag="tT")
            for h in range(2):
                ptT = p_tT.tile([128, 512], F32)
                for j in range(4):
                    n = 4 * h + j
                    nc.tensor.transpose(
                        ptT[:, 128 * j:128 * (j + 1)], t_all[:, n, :], identity
                    )
                dst = tT[:, 4 * h:4 * (h + 1), :].rearrange("p a b -> p (a b)")
                if h == 0:
                    nc.vector.tensor_copy(dst, ptT)
                else:
                    nc.scalar.copy(dst, ptT)

            # sum of squares over d via ACT square + POOL partition-add
            tT2 = work.tile([128, KT, 128], F32, tag="tT2")
            nc.scalar.square(tT2, tT)
            s_rep = srp.tile([128, KT, 128], F32, tag="s_rep")
            nc.gpsimd.partition_all_reduce(
                s_rep.rearrange("p a b -> p (a b)"),
                tT2.rearrange("p a b -> p (a b)"),
                channels=128, reduce_op=RED.add,
            )
            # layout gather row [1,(n,k)] -> [128(k), n] via DRAM bounce,
            # on the scalar/vector DMA queues to keep the sync queue free
            scr = dram.tile([1, K], F32, tag="scr")
            nc.scalar.dma_start(
                out=scr, in_=s_rep[0:1].rearrange("p a b -> p (a b)")
            )
            tsum = small.tile([128, KT], F32, tag="tsum")
            nc.scalar.dma_start(
                out=tsum, in_=scr.rearrange("o (n p) -> (o p) n", p=128)
            )
            tnrm = small.tile([128, KT], F32, tag="tnrm")
            nc.scalar.sqrt(tnrm, tsum)
            invt = small.tile([128, KT], F32, tag="invt")
            nc.vector.reciprocal(invt, tnrm)
            invt_of[vb] = invt

            # dots (raw tokens x normalized interests)
            pdots = p_dots.tile([128, KT, M], F32)
            iT = iT_of(b)  # noqa: uses real batch index
            for n in range(KT):
                nc.tensor.matmul(
                    pdots[:, n, :], lhsT=tT[:, n, :], rhs=iT,
                    start=True, stop=True,
                )
            pdots_of[vb] = pdots

        def tail(bb):
            s2 = bb % STG
            g = bb // STG
            if s2 == 0:
                st_t_new = stage.tile([128, STG, KT], F32, tag="st_t")
                st_i_new = stage.tile([128, STG, M], F32, tag="st_i")
                st_of[g] = (st_t_new, st_i_new)
            st_t, st_i = st_of[g]
            pd = pdots_of.pop(bb)
            invt = invt_of.pop(bb)
            # fused normalize + evacuate
            dn = work.tile([128, KT, M], F32, tag="dn")
            nc.vector.tensor_mul(
                dn, pd, invt.broadcast_to([128, KT, M])
            )
            # per-token max over m
            nc.vector.tensor_reduce(st_t[:, s2, :], dn, axis=AX.X, op=OP.max)
            # per-interest: max over n (DVE), then partitions (POOL)
            nmax = small.tile([128, M], F32, tag="nmax")
            nc.vector.tensor_reduce(
                nmax, dn.rearrange("p n m -> p m n"), axis=AX.X, op=OP.max
            )
            nc.gpsimd.partition_all_reduce(
                st_i[:, s2, :], nmax, channels=128, reduce_op=RED.max
            )
            if s2 == STG - 1:
                del st_of[g]
                dts = stage.tile([128, STG * KT], F32, tag="dts")
                nc.scalar.activation(
                    dts, st_t.rearrange("p a b -> p (a b)"),
                    ACT.Sqrt, bias=two[:], scale=-2.0,
                )
                nc.gpsimd.tensor_add(acc_t, acc_t, dts)
                dis = stage.tile([1, STG * M], F32, tag="dis")
                nc.scalar.activation(
                    dis, st_i[0:1].rearrange("o a b -> o (a b)"),
                    ACT.Sqrt, bias=two[:1], scale=-2.0,
                )
                nc.gpsimd.tensor_add(acc_i, acc_i, dis)

        nvb = b_loc * reps
        for vb in range(nvb + LAG):
            if vb < nvb:
                front(vb)
            if vb >= LAG:
                tail(vb - LAG)

        # ---------- final reductions ----------
        red_t = singles.tile([128, 1], F32)
        nc.vector.tensor_reduce(red_t, acc_t, axis=AX.X, op=OP.add)
        pfin = p_dots.tile([128, M], F32, tag="pdots")
        nc.tensor.matmul(pfin[:1, :1], lhsT=ones, rhs=red_t, start=True, stop=True)
        out_sb = small.tile([1, 2], F32, tag="out_sb")
        nc.scalar.copy(out_sb[:, 0:1], pfin[:1, :1])
        nc.vector.tensor_reduce(out_sb[:, 1:2], acc_i, axis=AX.X, op=OP.add)
        nc.sync.dma_start(out=out, in_=out_sb)

    nc.compile()
    return nc


_NC_CACHE = None


def _get_nc():
    global _NC_CACHE
    if _NC_CACHE is None:
        _NC_CACHE = build()
    return _NC_CACHE


def kernel(tokens: np.ndarray, interests: np.ndarray, _trace=False) -> np.ndarray:
    tokens = np.ascontiguousarray(tokens, dtype=np.float32)
    interests = np.ascontiguousarray(interests, dtype=np.float32)
    assert tokens.shape == (B, K, D) and interests.shape == (B, M, D)

    nc = _get_nc()
    in_maps = [
        {
            "tokens": tokens[c * B_LOC:(c + 1) * B_LOC],
            "interests": interests[c * B_LOC:(c + 1) * B_LOC],
        }
        for c in range(N_CORES)
    ]
    res = run_bass_kernel_spmd(
        nc, in_maps, core_ids=list(range(N_CORES)), trace=_trace
    )
    sum_t = 0.0  # sum over all (b, k) of min_m dist
    sum_i = 0.0  # sum over all (b, m) of min_k dist
    for r in res.results:
        sum_t += float(r["out"][0, 0])
        sum_i += float(r["out"][0, 1])
    loss = sum_i / (B * M) + ALPHA_T_TO_I * sum_t / (B * K)
    kernel.last_results = res
    return np.array(loss, dtype=np.float32)


# revision 54
# speedup vs baseline: 15028.7662x; 15018.4554x over previous
"""Chamfer loss kernel for TRN2 (8 NeuronCores, data-parallel over batch).

Reference computation (per batch b):
  t = l2_normalize(tokens[b])      # (K=1024, D=128)
  i = l2_normalize(interests[b])   # (M=64,  D=128)
  dist[k,m] = sqrt(2 - 2*dot(t_k, i_m))   (since ||t||=||i||=1)
  loss = mean_bm(min_k dist) + 0.3 * mean_bk(min_m dist)

min dist <=> max dot: reduce max over normalized dots, apply sqrt(2-2x)
only to tiny reduced tensors.

Structure (per core, 64 batches):
  phase 0: bulk-normalize+transpose ALL interests -> persistent iT_all
  per batch:
    DMA  tokens[b] -> t_all [128,(8,128)]
    PE   8 transposes (raw) -> psum -> copies -> tT [128d,(8n,128k)]
    PE   8 dots matmuls: pdots[k, (n m)] = tT.T @ iT_b
    ACT  tT2 = tT^2 (one big op)
    POOL partition-add over d -> sumsq per token (free-indexed row)
    DMA  4KB layout-gather: row -> tsum [128,8] (partition-indexed)
    ACT  sqrt, DVE reciprocal -> invt [128,8]
    DVE  dn = pdots * invt (fused normalize + psum evacuation)
    DVE  max over m -> staged [128,8]; max over n -> nmax [128,64]
    POOL partition-max over token partitions -> staged per-interest max
    every 4 batches: ACT sqrt(2-2x) on staged maxes, POOL accumulate
Host combines the 8 per-core partial sums.
"""

import os
import numpy as np
from contextlib import ExitStack

import concourse.bass as bass
import concourse.bass_isa as bass_isa
import concourse.mybir as mybir
import concourse.tile as tile
from concourse import bacc
from concourse.bass_utils import run_bass_kernel_spmd

N_CORES = 8
B, K, M, D = 512, 1024, 64, 128
B_LOC = B // N_CORES          # 64 batches per core
KT = K // 128                 # 8 token tiles of [128, D] per batch
ALPHA_T_TO_I = 0.3
STG = 4                       # sqrt-staging factor (batches per sqrt op)

F32 = mybir.dt.float32
AX = mybir.AxisListType
OP = mybir.AluOpType
ACT = mybir.ActivationFunctionType
RED = bass_isa.ReduceOp


def build(b_loc=B_LOC, reps=1):
    assert b_loc % 2 == 0 and b_loc % STG == 0
    nc = bacc.Bacc(
        "TRN2",
        target_bir_lowering=False,
        debug=False,
        num_devices=N_CORES,
    )
    tokens = nc.dram_tensor("tokens", [b_loc, K, D], F32, kind="ExternalInput").ap()
    interests = nc.dram_tensor(
        "interests", [b_loc, M, D], F32, kind="ExternalInput"
    ).ap()
    out = nc.dram_tensor("out", [1, 2], F32, kind="ExternalOutput").ap()

    NG = b_loc * M // 128     # interest row-groups of 128 (b_loc/2)

    with ExitStack() as ctx:
        tc = ctx.enter_context(tile.TileContext(nc))
        singles = ctx.enter_context(tc.tile_pool(name="singles", bufs=1))
        tok_pool = ctx.enter_context(tc.tile_pool(name="tok", bufs=5))
        work = ctx.enter_context(tc.tile_pool(name="work", bufs=5))
        small = ctx.enter_context(tc.tile_pool(name="small", bufs=10))
        stage = ctx.enter_context(tc.tile_pool(name="stage", bufs=4))
        srp = ctx.enter_context(tc.tile_pool(name="srp", bufs=4))
        p_tT = ctx.enter_context(tc.tile_pool(name="p_tT", bufs=3, space="PSUM"))
        p_dots = ctx.enter_context(tc.tile_pool(name="p_dots", bufs=5, space="PSUM"))
        dram = ctx.enter_context(tc.tile_pool(name="dram", bufs=8, space="DRAM"))

        identity = singles.tile([128, 128], F32)
        nc.gpsimd.memset(identity, 0.0)
        nc.gpsimd.affine_select(
            out=identity, in_=identity, compare_op=OP.not_equal, fill=1.0,
            base=0, pattern=[[-1, 128]], channel_multiplier=1,
        )
        ones = singles.tile([128, 1], F32)
        nc.vector.memset(ones, 1.0)
        two = singles.tile([128, 1], F32)
        nc.vector.memset(two, 2.0)
        acc_t = singles.tile([128, STG * KT], F32)
        acc_i = singles.tile([1, STG * M], F32)
        nc.vector.memset(acc_t, 0.0)
        nc.vector.memset(acc_i, 0.0)

        # ---------- phase 0: all interests -> normalized iT_all ----------
        # interests flat (b*M, D) -> groups of 128 rows
        i_flat = interests.rearrange("b m d -> (b m) d").rearrange(
            "(g p) d -> p g d", p=128
        )  # [128, NG, 128]
        i_all = singles.tile([128, NG, D], F32)
        nc.sync.dma_start(out=i_all, in_=i_flat)
        isum = singles.tile([128, NG], F32)
        itrash = work.tile([128, D], F32, tag="trash")
        for g in range(NG):
            nc.scalar.activation(
                itrash, i_all[:, g, :], ACT.Square, accum_out=isum[:, g:g + 1]
            )
        inrm = singles.tile([128, NG], F32)
        nc.scalar.sqrt(inrm, isum)
        invi = singles.tile([128, NG], F32)
        nc.vector.reciprocal(invi, inrm)
        nc.vector.tensor_mul(i_all, i_all, invi.broadcast_to([128, NG, D]))
        iT_all = singles.tile([128, NG, 128], F32)   # [d, (g, bm)]
        for h0 in range(0, NG, 4):
            cn = min(4, NG - h0)
            piT = p_tT.tile([128, 512], F32, tag="ptT")
            for j in range(cn):
                g = h0 + j
                nc.tensor.transpose(
                    piT[:, 128 * j:128 * (j + 1)], i_all[:, g, :], identity
                )
            dst = iT_all[:, h0:h0 + cn, :].rearrange("p a b -> p (a b)")
            if (h0 // 4) % 2 == 0:
                nc.vector.tensor_copy(dst, piT[:, :128 * cn])
            else:
                nc.scalar.copy(dst, piT[:, :128 * cn])

        def iT_of(b):
            # batch b's interests: rows (b*M..b*M+M) = group b//2, half b%2
            return iT_all[:, b // 2, (b % 2) * M:(b % 2) * M + M]

        # ---------- main loop (software-pipelined: tail lags front by STG) ----------
        pdots_of = {}
        tsum_of = {}
        invt_of = {}
        st_of = {}
        LAG = 4

        def front(vb):
            b = vb % b_loc
            t_all = tok_pool.tile([128, KT, D], F32)
            nc.sync.dma_start(
                out=t_all, in_=tokens[b].rearrange("(n p) d -> p n d", p=128)
            )

            # transposes of raw token tiles
            tT = work.tile([128, KT, 128], F32, tag="tT")
            for h in range(2):
                ptT = p_tT.tile([128, 512], F32)
                for j in range(4):
                    n = 4 * h + j
                    nc.tensor.transpose(
                        ptT[:, 128 * j:128 * (j + 1)], t_all[:, n, :], identity
                    )
                dst = tT[:, 4 * h:4 * (h + 1), :].rearrange("p a b -> p (a b)")
                if h == 0:
                    nc.vector.tensor_copy(dst, ptT)
                else:
                    nc.scalar.copy(dst, ptT)

            # sum of squares over d via ACT square + POOL partition-add
            tT2 = work.tile([128, KT, 128], F32, tag="tT2")
            nc.scalar.square(tT2, tT)
            s_rep = srp.tile([128, KT, 128], F32, tag="s_rep")
            nc.gpsimd.partition_all_reduce(
                s_rep.rearrange("p a b -> p (a b)"),
                tT2.rearrange("p a b -> p (a b)"),
                channels=128, reduce_op=RED.add,
            )
            # layout gather row [1,(n,k)] -> [128(k), n] via DRAM bounce,
            # on the scalar/vector DMA queues to keep the sync queue free
            scr = dram.tile([1, K], F32, tag="scr")
            nc.scalar.dma_start(
                out=scr, in_=s_rep[0:1].rearrange("p a b -> p (a b)")
            )
            tsum = small.tile([128, KT], F32, tag="tsum")
            nc.scalar.dma_start(
                out=tsum, in_=scr.rearrange("o (n p) -> (o p) n", p=128)
            )
            tsum_of[vb] = tsum

            # dots (raw tokens x normalized interests)
            pdots = p_dots.tile([128, KT, M], F32)
            iT = iT_of(b)  # noqa: uses real batch index
            for n in range(KT):
                nc.tensor.matmul(
                    pdots[:, n, :], lhsT=tT[:, n, :], rhs=iT,
                    start=True, stop=True,
                )
            pdots_of[vb] = pdots

        def tail(bb):
            s2 = bb % STG
            g = bb // STG
            if s2 == 0:
                st_t_new = stage.tile([128, STG, KT], F32, tag="st_t")
                st_i_new = stage.tile([128, STG, M], F32, tag="st_i")
                st_of[g] = (st_t_new, st_i_new)
            st_t, st_i = st_of[g]
            pd = pdots_of.pop(bb)
            tnrm = small.tile([128, KT], F32, tag="tnrm")
            nc.scalar.sqrt(tnrm, tsum_of.pop(bb))
            invt = small.tile([128, KT], F32, tag="invt")
            nc.vector.reciprocal(invt, tnrm)
            # fused normalize + evacuate
            dn = work.tile([128, KT, M], F32, tag="dn")
            nc.vector.tensor_mul(
                dn, pd, invt.broadcast_to([128, KT, M])
            )
            # per-token max over m
            nc.vector.tensor_reduce(st_t[:, s2, :], dn, axis=AX.X, op=OP.max)
            # per-interest: max over n (DVE), then partitions (POOL)
            nmax = small.tile([128, M], F32, tag="nmax")
            nc.vector.tensor_reduce(
                nmax, dn.rearrange("p n m -> p m n"), axis=AX.X, op=OP.max
            )
            nc.gpsimd.partition_all_reduce(
                st_i[:, s2, :], nmax, channels=128, reduce_op=RED.max
            )
            if s2 == STG - 1:
                del st_of[g]
                dts = stage.tile([128, STG * KT], F32, tag="dts")
                nc.scalar.activation(
                    dts, st_t.rearrange("p a b -> p (a b)"),
                    ACT.Sqrt, bias=two[:], scale=-2.0,
                )
                nc.gpsimd.tensor_add(acc_t, acc_t, dts)
                dis = stage.tile([1, STG * M], F32, tag="dis")
                nc.scalar.activation(
                    dis, st_i[0:1].rearrange("o a b -> o (a b)"),
                    ACT.Sqrt, bias=two[:1], scale=-2.0,
                )
                nc.gpsimd.tensor_add(acc_i, acc_i, dis)

        nvb = b_loc * reps
        for vb in range(nvb + LAG):
            if vb < nvb:
                front(vb)
            if vb >= LAG:
                tail(vb - LAG)

        # ---------- final reductions ----------
        red_t = singles.tile([128, 1], F32)
        nc.vector.tensor_reduce(red_t, acc_t, axis=AX.X, op=OP.add)
        pfin = p_dots.tile([128, M], F32, tag="pdots")
        nc.tensor.matmul(pfin[:1, :1], lhsT=ones, rhs=red_t, start=True, stop=True)
        out_sb = small.tile([1, 2], F32, tag="out_sb")
        nc.scalar.copy(out_sb[:, 0:1], pfin[:1, :1])
        nc.vector.tensor_reduce(out_sb[:, 1:2], acc_i, axis=AX.X, op=OP.add)
        nc.sync.dma_start(out=out, in_=out_sb)

    nc.compile()
    return nc


_NC_CACHE = None


def _get_nc():
    global _NC_CACHE
    if _NC_CACHE is None:
        _NC_CACHE = build()
    return _NC_CACHE


def kernel(tokens: np.ndarray, interests: np.ndarray, _trace=False) -> np.ndarray:
    tokens = np.ascontiguousarray(tokens, dtype=np.float32)
    interests = np.ascontiguousarray(interests, dtype=np.float32)
    assert tokens.shape == (B, K, D) and interests.shape == (B, M, D)

    nc = _get_nc()
    in_maps = [
        {
            "tokens": tokens[c * B_LOC:(c + 1) * B_LOC],
            "interests": interests[c * B_LOC:(c + 1) * B_LOC],
        }
        for c in range(N_CORES)
    ]
    res = run_bass_kernel_spmd(
        nc, in_maps, core_ids=list(range(N_CORES)), trace=_trace
    )
    sum_t = 0.0  # sum over all (b, k) of min_m dist
    sum_i = 0.0  # sum over all (b, m) of min_k dist
    for r in res.results:
        sum_t += float(r["out"][0, 0])
        sum_i += float(r["out"][0, 1])
    loss = sum_i / (B * M) + ALPHA_T_TO_I * sum_t / (B * K)
    kernel.last_results = res
    return np.array(loss, dtype=np.float32)
